# revision 1
# baseline (speedup 1.0000x reference)
"""Chamfer loss kernel for Trainium2 (8 NeuronCores).

Problem: pred [4, 8192, 3], target [4, 8192, 3] ->
    scalar = mean_b( mean_n min_m dist(pred_bn, target_bm)
           + mean_m min_n dist(pred_bn, target_bm) )

Strategy
--------
* 8 cores = 4 batches x 2 pred-halves.  Core (2b+h) owns pred rows
  [h*4096, (h+1)*4096) of batch b and all 8192 targets, computing both
  reduction directions of its half of the distance matrix:
    - dist1 (min over targets) for its 4096 pred points  (complete)
    - dist2 partial (min over its pred points) for all 8192 targets
      (host combines the two halves with an elementwise min)
* Distances are computed directly on the VectorEngine in the stable
  form  s = -((tx-px)^2 + (ty-py)^2 + (tz-pz)^2)  using two custom
  fused DVE ops per 128-pred row tile over the full 8192-target free
  dim (pred coords enter as per-partition scalars, target coords as
  partition-broadcast rows):
      op A:  u = (Src0-C0)^2 + (Src1-C1)^2            (tx,px,ty,py)
      op B:  s = -(Src1 + (Src0-C0)^2), accum = max_k s  (tz,pz,u)
  op B's accumulator IS the row reduction (min dist^2, negated), so
  dist1 needs no separate reduce.  A native tensor_max folds s into a
  column accumulator for dist2; one gpsimd partition_all_reduce(max)
  finishes it.  Negation makes every reduction a MAX, which all native
  paths support (partition_all_reduce has max but not min).
* min/max commute with sqrt, so sqrt touches only the 4096+8192 final
  values per core (ACT evaluates sqrt(-x) in one pass via scale=-1).
* This layout minimizes instruction count (~105/core, vs ~650 for a
  matmul-based formulation), which is what the execution backend here
  is bound by: per-instruction dispatch dominates and a DVE
  instruction may cover 128x8192 elements.  Every distance is still
  produced and consumed exactly once on the VectorEngine.
* Host side only shards inputs, gathers per-core min-distance vectors
  and averages them (pure gather/unshard arithmetic).
"""

import numpy as np

_NPTS = 8192   # points per side (N == M)
_P = 128       # partitions per row tile


def _register_chamfer_ops():
    """Register the two fused distance ops (idempotent)."""
    import concourse.dve_ops as dve_ops
    from concourse.dve_spec import (
        AluOp, C0, C1, Spec, Src0, Src1, Zero, lower, minn, sq,
    )
    from concourse.dve_uop import DveOpSpec

    def make(name, spec):
        for op in dve_ops.OPS:
            if op.name == name:
                return op
        op = dve_ops.DveOp(name, spec, subdim=False, uops_sha={})
        dve_ops.OPS.append(op)
        row = dve_ops._CUSTOM_DVE_ROW_BASE + len(dve_ops.OPS) - 1
        assert row < 0x20, "custom DVE opcode row overflow"
        dve_ops._SUB_OPCODE_FOR_NAME[name] = row
        dve_ops.CUSTOM_DVE_SPECS[name] = spec
        for ver in ("v3", "v4"):
            s = DveOpSpec(
                name=name, opcode=row, uops=lower(spec, ver=ver),
                rd1_en=dve_ops.has_src1(spec),
            )
            op.uops_sha[ver] = s.sha(ver)
        return op

    def _ref_a(in0, in1, c0, c1, c2):
        a = in0.astype(np.float32) - c0
        b = in1.astype(np.float32) - c1
        return a * a + b * b

    def _ref_b(in0, in1, c0, c1, c2):
        a = in0.astype(np.float32) - c0
        s = np.minimum(-(in1.astype(np.float32) + a * a), 0.0)
        return s, s.reshape(s.shape[0], -1).max(axis=-1, keepdims=True)

    op_a = make(
        "CHAMFER_SQ2_ANT",
        Spec(body=sq(Src0 - C0) + sq(Src1 - C1), reference=_ref_a),
    )
    # clamped to <=0 in-op so sqrt(-x) downstream needs no separate relu
    op_b = make(
        "CHAMFER_SQ1NC_MAX_ANT",
        Spec(body=minn(-(Src1 + sq(Src0 - C0)), Zero), accum=AluOp.MAX,
             reference=_ref_b),
    )
    return op_a, op_b


def _build_kernel(n_pts=_NPTS // 2, m_pts=_NPTS, repeats=1):
    import concourse.bacc as bacc
    import concourse.bass as bass
    import concourse.bass_isa as bass_isa
    import concourse.mybir as mybir
    import concourse.tile as tile

    f32 = mybir.dt.float32
    n_rt = n_pts // _P       # row tiles (pred)
    op_a, op_b = _register_chamfer_ops()

    nc = bacc.Bacc("TRN2", target_bir_lowering=False, debug=False, num_devices=8)
    # pxyz[p, c, r] = pred coord c of point r*128+p ; txyz[c, m] = target coords
    pxyz_d = nc.dram_tensor("pxyz", [_P, 3, n_rt], f32, kind="ExternalInput")
    txyz_d = nc.dram_tensor("txyz", [3, m_pts], f32, kind="ExternalInput")
    d1_d = nc.dram_tensor("d1", [_P, n_rt], f32, kind="ExternalOutput")
    d2_d = nc.dram_tensor("d2", [1, m_pts], f32, kind="ExternalOutput")

    with tile.TileContext(nc) as tc:
        with (
            tc.tile_pool(name="const", bufs=1) as cpool,
            tc.tile_pool(name="work", bufs=1) as wpool,
        ):
            pxyz = cpool.tile([_P, 3, n_rt], f32)
            nc.sync.dma_start(pxyz[:], pxyz_d[:])
            # broadcast target coords to all partitions with one DMA
            txyz = cpool.tile([_P, 3, m_pts], f32)
            nc.sync.dma_start(
                txyz[:],
                bass.AP(txyz_d, 0, [[0, _P], [m_pts, 3], [1, m_pts]]),
            )
            part = cpool.tile([_P, n_rt], f32)   # max_m s  = -min dist^2 (<=0)
            d1 = cpool.tile([_P, n_rt], f32)

            for _ in range(repeats):
                cm = None
                for r in range(n_rt):
                    u = wpool.tile([_P, m_pts], f32, tag="u")
                    nc.vector._custom_dve(
                        op_a, out=u[:],
                        in0=txyz[:, 0, :], in1=txyz[:, 1, :],
                        s0=pxyz[:, 0, r:r + 1], s1=pxyz[:, 1, r:r + 1],
                    )
                    if cm is None:
                        cm = wpool.tile([_P, m_pts], f32, tag="cm")
                        s_out = cm
                    else:
                        s_out = u  # in-place: s overwrites u
                    nc.vector._custom_dve(
                        op_b, out=s_out[:],
                        in0=txyz[:, 2, :], in1=u[:],
                        s0=pxyz[:, 2, r:r + 1],
                        accum_out=part[:, r:r + 1],
                    )
                    if s_out is not cm:
                        nc.vector.tensor_max(cm[:], s_out[:], cm[:])

                # dist1 = sqrt(-part)   (part already clamped <= 0 in op_b)
                nc.scalar.activation(
                    d1[:], part[:], mybir.ActivationFunctionType.Sqrt, scale=-1.0
                )

                # dist2 = sqrt(-allreduce_max(cm))   (cm <= 0 likewise)
                ar = wpool.tile([_P, m_pts], f32, tag="u")
                nc.gpsimd.partition_all_reduce(
                    ar[:], cm[:], _P, bass_isa.ReduceOp.max
                )
                d2 = wpool.tile([1, m_pts], f32, tag="cm")
                nc.scalar.activation(
                    d2[:], ar[0:1, :], mybir.ActivationFunctionType.Sqrt,
                    scale=-1.0,
                )

            nc.sync.dma_start(d1_d[:], d1[:])
            nc.sync.dma_start(d2_d[:], d2[:])

    nc.compile()
    return nc


_NC_CACHE = None
_LAST_RESULT = None  # BassKernelResults of the most recent run (for test harness)


def _get_nc():
    global _NC_CACHE
    if _NC_CACHE is None:
        _NC_CACHE = _build_kernel()
    return _NC_CACHE


def _pxyz(pts):
    """[n, 3] f32 -> [128, 3, n//128] per-partition scalar layout."""
    n = pts.shape[0]
    return np.ascontiguousarray(
        pts.astype(np.float32).reshape(n // _P, _P, 3).transpose(1, 2, 0)
    )


def kernel(pred, target):
    from concourse.bass_utils import run_bass_kernel_spmd

    pred = np.asarray(pred, dtype=np.float32)
    target = np.asarray(target, dtype=np.float32)
    B = pred.shape[0]
    half = pred.shape[1] // 2

    in_maps = []
    for b in range(B):
        txyz = np.ascontiguousarray(target[b].astype(np.float32).T)
        for h in range(2):
            in_maps.append({
                "pxyz": _pxyz(pred[b, h * half:(h + 1) * half]),
                "txyz": txyz,
            })

    nc = _get_nc()
    res = run_bass_kernel_spmd(nc, in_maps, list(range(2 * B)))
    global _LAST_RESULT
    _LAST_RESULT = res

    total = 0.0
    for b in range(B):
        d1a = res.results[2 * b]["d1"]       # [128, 32] dist1, pred rows 0..4095
        d1b = res.results[2 * b + 1]["d1"]   # [128, 32] dist1, pred rows 4096..
        d2a = res.results[2 * b]["d2"][0]    # [8192] dist2 partial (pred half a)
        d2b = res.results[2 * b + 1]["d2"][0]
        ch1 = 0.5 * (d1a.mean(dtype=np.float64) + d1b.mean(dtype=np.float64))
        ch2 = np.minimum(d2a, d2b).mean(dtype=np.float64)
        total += ch1 + ch2
    return np.float32(total / B)

